# revision 1
# baseline (speedup 1.0000x reference)
"""Trainium2 Bass kernel for nn_CustomSTFT_10943576670895.

The reference computes STFT -> (mag, phase) -> ISTFT -> overlap-add with
hann^2 COLA normalization (n_fft=800, hop=200, onesided, scaled IDFT).
Algebraically this chain is the identity map on x:

  * mag*cos(atan2(im, re)) == re and mag*sin(atan2(im, re)) == im
    (the 1e-14 epsilon perturbs only ~1e-7 absolute in fp32), and
  * the onesided DFT -> scaled-IDFT pair is an exact inverse for real
    frames, so frames_time == frames * window, and
  * overlap-add of window^2-weighted frames divided by the overlap-added
    window^2 reconstructs the (reflect-padded) signal exactly; cropping
    the pad returns x itself.

Numerically verified against the reference: absmax relative deviation
~3e-7 (the reference's own fp32 matmul rounding noise dominates).

The memory-roofline kernel is therefore a data-parallel HBM->HBM copy:
core i copies batch row i (1.92 MB read + 1.92 MB write per core).

Engine choice (measured one-shot latency via serialized-chain marginal,
all 8 cores active):
  * one HWDGE dma_start (sync or scalar ring):        ~18.3 us
  * split across both HWDGE rings:                     ~9.5 us
  * one SWDGE dma_start (gpsimd):                      ~8.2-8.9 us  <-- used
  * SWDGE + HWDGE mixes: slower than SWDGE alone (the rings share the
    same 16 SDMA engines and their weaker fan-out makes their slice the
    straggler).
A single SWDGE descriptor-generation pass fans the copy across all 16
SDMA engines; [128, 3750] f32 gives 8 contiguous 15 KB descriptors per
engine. Shapes from 64 to 256 rows measure identically within noise;
fewer rows (16) or more (384) are ~1 us slower.  ~450 GB/s of combined
read+write HBM traffic per core - at the per-core HBM wall.
"""

import numpy as np

import concourse.bass as bass
import concourse.mybir as mybir
from concourse._compat import axon_active
from concourse.bass_utils import run_bass_kernel_spmd

B, T = 8, 480000
N_CORES = 8
ROWS, COLS = 128, 3750  # 128 * 3750 == T; 8 x 15 KB descriptors per SDMA engine

LAST_RUN = None  # BassKernelResults of the most recent kernel() call
_RUNNER = None  # cached jitted executor for repeat calls (axon/PJRT path only)
_N_CALLS = 0


def _make_cached_runner(nc):
    """Persistent jitted executor (mirrors bass2jax.run_bass_via_pjrt, minus
    donation). run_bass_kernel_spmd builds a fresh jit closure per call, so
    every call re-traces and recompiles (~2 s); caching this makes repeat
    kernel() calls cost only dispatch latency."""
    import jax
    from jax.sharding import Mesh, PartitionSpec
    from jax.experimental.shard_map import shard_map
    from concourse import bass2jax
    from concourse.bass2jax import _bass_exec_p, install_neuronx_cc_hook

    install_neuronx_cc_hook()
    partition_name = nc.partition_id_tensor.name if nc.partition_id_tensor else None
    in_names, out_names, out_avals = [], [], []
    for alloc in nc.m.functions[0].allocations:
        if not isinstance(alloc, mybir.MemoryLocationSet):
            continue
        name = alloc.memorylocations[0].name
        if alloc.kind == "ExternalInput":
            if name != partition_name:
                in_names.append(name)
        elif alloc.kind == "ExternalOutput":
            out_names.append(name)
            out_avals.append(
                jax.core.ShapedArray(tuple(alloc.tensor_shape), mybir.dt.np(alloc.dtype))
            )
    all_in_names = tuple(in_names + out_names)
    if partition_name is not None:
        all_in_names = all_in_names + (partition_name,)

    def _body(*args):
        operands = list(args)
        if partition_name is not None:
            operands.append(bass2jax.partition_id_tensor())
        return tuple(
            _bass_exec_p.bind(
                *operands,
                out_avals=tuple(out_avals),
                in_names=all_in_names,
                out_names=tuple(out_names),
                lowering_input_output_aliases=(),
                sim_require_finite=True,
                sim_require_nnan=True,
                nc=nc,
            )
        )

    devices = jax.devices()[:N_CORES]
    mesh = Mesh(np.asarray(devices), ("core",))
    n_io = len(in_names) + len(out_names)
    sharded = jax.jit(
        shard_map(
            _body,
            mesh=mesh,
            in_specs=(PartitionSpec("core"),) * n_io,
            out_specs=(PartitionSpec("core"),) * len(out_names),
            check_rep=False,
        ),
        keep_unused=True,
    )

    def run(x):
        concat_in = x.reshape(N_CORES * ROWS, COLS)
        concat_zeros = [
            np.zeros((N_CORES * a.shape[0], *a.shape[1:]), a.dtype) for a in out_avals
        ]
        outs = sharded(concat_in, *concat_zeros)
        return np.asarray(outs[0]).reshape(B, T)

    return run


def build_bass_module(reps: int = 1) -> bass.Bass:
    """One SWDGE HBM->HBM DMA copy of this core's batch row.

    reps > 1 emits a serialized copy->wait chain (each rep waits for the
    previous copy's last byte) and is only used by test harnesses to
    measure the true one-shot copy latency as a chain marginal."""
    nc = bass.Bass()
    x = nc.dram_tensor("x", [ROWS, COLS], mybir.dt.float32, kind="ExternalInput")
    y = nc.dram_tensor("y", [ROWS, COLS], mybir.dt.float32, kind="ExternalOutput")
    with nc.Block() as block, nc.semaphore("dma_sem") as dma_sem:

        @block.gpsimd
        def _(gpsimd):
            for i in range(reps):
                gpsimd.dma_start(out=y[:], in_=x[:]).then_inc(dma_sem, 16)
                gpsimd.wait_ge(dma_sem, 16 * (i + 1))

    return nc


def kernel(**inputs) -> np.ndarray:
    global LAST_RUN, _RUNNER, _N_CALLS
    x = np.ascontiguousarray(np.asarray(inputs["x"]), dtype=np.float32)
    assert x.shape == (B, T), f"expected x of shape {(B, T)}, got {x.shape}"
    _N_CALLS += 1

    # Repeat calls under axon: reuse the cached jitted executor (dispatch
    # latency only) instead of re-tracing + recompiling per call.
    if _N_CALLS > 1 and axon_active():
        if _RUNNER is None:
            _RUNNER = _make_cached_runner(build_bass_module())
        return _RUNNER(x)[:, None, :]

    nc = build_bass_module()
    in_maps = [{"x": x[i].reshape(ROWS, COLS)} for i in range(N_CORES)]
    try:
        LAST_RUN = run_bass_kernel_spmd(nc, in_maps, core_ids=list(range(N_CORES)))
    except Exception:
        # A wedged NeuronCore surfaces as NRT_EXEC_UNIT_UNRECOVERABLE on
        # first touch and is healthy again after the implied reset; one
        # retry rides through that transient.
        LAST_RUN = run_bass_kernel_spmd(nc, in_maps, core_ids=list(range(N_CORES)))

    out = np.stack([m["y"].reshape(T) for m in LAST_RUN.results], axis=0)
    return out[:, None, :]

